# revision 11
# baseline (speedup 1.0000x reference)
"""Distributed 2-hop GCN (scatter-mean propagation) for 8 Trainium2 NeuronCores.

Math:  h = x @ W.T (+ b folded to the end since mean-propagation of a
constant row is the same constant row);  two hops of
h <- segment_mean(h[dst], src) over edges with self loops.

Scheme (per core, nodes sharded by target):
  * host sorts the core's incoming edges by target and greedily packs them
    into "tiles" of 128 edge slots containing at most K=8 whole targets;
  * per tile one indirect DMA gathers the 128 source rows (messages) from
    the allgathered node table;
  * Sel[p, k] = (seg[p] == k) is built on DVE from host metadata; one PE
    matmul  msgs^T @ Sel  accumulates each target's sum into PSUM columns
    ("slots"); 64 tiles share one [64, 512] PSUM bank;
  * banks are transposed back to row-major via PE and written to a local
    slot table; a compaction pass gathers each target's slot row, scales by
    1/deg (and adds b on the final hop);
  * compacted shards are AllGathered between hops.
"""

import numpy as np

N = 100000
NCORES = 8
SHARD = N // NCORES           # 12500
GROUPS = 98                   # 98 * 128 = 12544
SHARD_PAD = GROUPS * 128      # 12544
F = 64                        # output features
IN = 128                      # input features
P = 128                       # edge slots per tile
K = 8                         # target slots per tile
TB = 64                       # tiles per chunk (TB*K = 512 = one PSUM bank)
NUM_LAYERS = 2
NQ = 4                        # SWDGE queues for indirect gathers
MSG_BUFS = 16                 # rotation depth for per-tile message tiles


# ----------------------------------------------------------------------------
# host-side preprocessing
# ----------------------------------------------------------------------------

def _pack_core(src_c, dst_c):
    """Edges with target src_c (within one core's node range), sources dst_c.
    Greedy pack into tiles of P edge slots / K whole targets.
    Returns gsrc [T,P] int64 (source node id per edge slot), seg [T,P] f32
    (slot id, K = unused), slot_tgt [T*K] int64 (target node per slot, -1
    dummy)."""
    order = np.argsort(src_c, kind="stable")
    s = src_c[order]
    d = dst_c[order]
    tgt_ids, seg_starts = np.unique(s, return_index=True)
    seg_ends = np.append(seg_starts[1:], len(s))
    degs = seg_ends - seg_starts
    # bin-pack by degree: big-small pairing maximizes tile fill
    bydeg = np.argsort(degs, kind="stable")
    lo, hi = 0, len(bydeg) - 1
    tiles_g, tiles_s, slot_tgt = [], [], []
    cur_g = np.zeros(P, np.int64)
    cur_s = np.full(P, K, np.float32)
    fill = 0
    slots = 0

    def flush():
        nonlocal fill, slots
        slot_tgt.extend([-1] * (K - slots))
        tiles_g.append(cur_g.copy())
        tiles_s.append(cur_s.copy())
        cur_g[:] = 0
        cur_s[:] = K
        fill = 0
        slots = 0

    def put(ti):
        nonlocal fill, slots
        deg = int(degs[ti])
        a = int(seg_starts[ti])
        cur_g[fill:fill + deg] = d[a:a + deg]
        cur_s[fill:fill + deg] = slots
        slot_tgt.append(int(tgt_ids[ti]))
        fill += deg
        slots += 1

    while lo <= hi:
        # place the largest remaining, then fill with smallest ones
        if fill + int(degs[bydeg[hi]]) <= P and slots < K:
            put(int(bydeg[hi]))
            hi -= 1
        elif fill + int(degs[bydeg[lo]]) <= P and slots < K:
            put(int(bydeg[lo]))
            lo += 1
        else:
            flush()
    if slots or fill:
        flush()
    return (np.stack(tiles_g), np.stack(tiles_s),
            np.array(slot_tgt, np.int64))


def _node_to_row(v):
    """node id -> row in the allgathered (padded-shard) node table."""
    c = v // SHARD
    return c * SHARD_PAD + (v - c * SHARD)


def _prepare(edge_index):
    src = np.asarray(edge_index[0], np.int64)
    dst = np.asarray(edge_index[1], np.int64)
    loops = np.arange(N, dtype=np.int64)
    deg = (np.bincount(src, minlength=N) + 1).astype(np.float32)
    s_all = src
    d_all = dst

    packed = []
    for c in range(NCORES):
        base = c * SHARD
        m = (s_all >= base) & (s_all < base + SHARD)
        packed.append(_pack_core(s_all[m], d_all[m]))
    tmax = max(p[0].shape[0] for p in packed) + 1
    nchunk = -(-tmax // TB)
    tpad = nchunk * TB

    cores = []
    for c in range(NCORES):
        g, sgm, st = packed[c]
        t = g.shape[0]
        if t < tpad:
            g = np.concatenate([g, np.zeros((tpad - t, P), np.int64)])
            sgm = np.concatenate([sgm, np.full((tpad - t, P), K, np.float32)])
            st = np.concatenate([st, np.full((tpad - t) * K, -1, np.int64)])
        # gather indices in node-table rows, laid out [nchunk, 128, TB]
        gi = _node_to_row(g).astype(np.int32)          # [tpad, P]
        gidxT = gi.reshape(nchunk, TB, P).transpose(0, 2, 1).copy()
        segT = sgm.reshape(nchunk, TB, P).transpose(0, 2, 1).astype(np.float32).copy()
        # compaction: local target t -> its slot row in the local slot table;
        # targets with no packed edges (self-loop only) and pad targets point
        # at a dummy (all-zero) slot.
        dummies = np.nonzero(st < 0)[0]
        assert dummies.size > 0, "no dummy slot available"
        slot_of = np.full(SHARD_PAD, dummies[0], np.int64)
        valid = st >= 0
        slot_of[st[valid] - c * SHARD] = np.nonzero(valid)[0]
        cidxT = slot_of.reshape(GROUPS, P).T.astype(np.int32).copy()   # [128, GROUPS]
        ideg = np.zeros(SHARD_PAD, np.float32)
        ideg[:SHARD] = 1.0 / np.maximum(deg[c * SHARD:(c + 1) * SHARD], 1.0)
        idegT = ideg.reshape(GROUPS, P).T.copy()                       # [128, GROUPS]
        cores.append(dict(gidxT=gidxT, segT=segT, cidxT=cidxT, idegT=idegT,
                          gsrc=g))
    return cores, nchunk


# ----------------------------------------------------------------------------
# walrus workaround: this compiler build accepts at most ONE sync-wait per
# instruction; move extra waits onto preceding NoOps on the same engine.
# ----------------------------------------------------------------------------

def _split_sync_waits(nc, mybir):
    n = 0
    for f in nc.m.functions:
        for b in f.blocks:
            out = []
            changed = False
            for inst in b.instructions:
                si = inst.sync_info
                waits = list(si.on_wait) if si is not None and si.on_wait else []
                if len(waits) > 1:
                    changed = True
                    for w in waits[:-1]:
                        nop = mybir.InstNoOp(
                            name=f"wsplit_{b.name}_{n}", ins=[], outs=[],
                            engine=inst.engine,
                        )
                        n += 1
                        nop.sync_info = mybir.SyncInfo(on_wait=[w], on_update=[])
                        out.append(nop)
                    si.on_wait = waits[-1:]
                out.append(inst)
            if changed:
                b.instructions = out
    return n


def _patch_bass(bass, mybir):
    if getattr(bass.Bass, "_gcn_wait_patch", False):
        return
    orig = bass.Bass.to_json_bytes

    def patched(self, *a, **k):
        _split_sync_waits(self, mybir)
        return orig(self, *a, **k)

    bass.Bass.to_json_bytes = patched
    bass.Bass._gcn_wait_patch = True


# ----------------------------------------------------------------------------
# bass program
# ----------------------------------------------------------------------------

def _build(nchunk, repeat=1):
    import concourse.bass as bass
    import concourse.mybir as mybir
    from concourse.tile import TileContext

    _patch_bass(bass, mybir)

    nslot = nchunk * TB * K
    dt = mybir.dt
    nc = bass.Bass(num_swdge_queues=NQ)

    xsh = nc.dram_tensor("xsh", [SHARD_PAD, IN], dt.float32, kind="ExternalInput")
    xg = nc.dram_tensor("xg", [nchunk, 128, TB * IN], dt.float32, kind="ExternalInput")
    wt = nc.dram_tensor("wt", [IN, F], dt.float32, kind="ExternalInput")
    bb = nc.dram_tensor("bb", [128, F], dt.float32, kind="ExternalInput")
    id128 = nc.dram_tensor("id128", [128, 128], dt.float32, kind="ExternalInput")
    id64 = nc.dram_tensor("id64", [64, 64], dt.float32, kind="ExternalInput")
    iota = nc.dram_tensor("iota", [128, TB * K], dt.float32, kind="ExternalInput")
    gidxT = nc.dram_tensor("gidxT", [nchunk, 128, TB], dt.int32, kind="ExternalInput")
    segT = nc.dram_tensor("segT", [nchunk, 128, TB], dt.float32, kind="ExternalInput")
    cidxT = nc.dram_tensor("cidxT", [128, GROUPS], dt.int32, kind="ExternalInput")
    idegT = nc.dram_tensor("idegT", [128, GROUPS], dt.float32, kind="ExternalInput")
    out = nc.dram_tensor("out", [SHARD_PAD, F], dt.float32, kind="ExternalOutput")

    h_loc = [nc.dram_tensor(f"h{i}_loc", [SHARD_PAD, F], dt.float32)
             for i in range(NUM_LAYERS)]
    h_glob = [None if i == 0 else
              nc.dram_tensor(f"h{i}_glob", [NCORES * SHARD_PAD, F], dt.float32,
                             addr_space="Shared")
              for i in range(NUM_LAYERS)]
    slots = [nc.dram_tensor(f"slots{i}", [nslot, F], dt.float32)
             for i in range(NUM_LAYERS)]

    with TileContext(nc) as tc:
        import contextlib
        with contextlib.ExitStack() as ctx:
            cpool = ctx.enter_context(tc.tile_pool(name="consts", bufs=1))
            mpool = ctx.enter_context(tc.tile_pool(name="msgs", bufs=MSG_BUFS))
            spool = ctx.enter_context(tc.tile_pool(name="sel", bufs=3))
            ipool = ctx.enter_context(tc.tile_pool(name="idx", bufs=3))
            tpool = ctx.enter_context(tc.tile_pool(name="stage", bufs=8))
            ppool = ctx.enter_context(tc.tile_pool(name="psum", bufs=2, space="PSUM"))
            qpool = ctx.enter_context(tc.tile_pool(name="psum_t", bufs=2, space="PSUM"))
            xpool = ctx.enter_context(tc.tile_pool(name="psum_x", bufs=2, space="PSUM"))
            xgpool = ctx.enter_context(tc.tile_pool(name="xgbuf", bufs=2))

            _qn = [0]

            def _indirect(out_ap, in_tensor, idx_ap):
                inst = nc.gpsimd.indirect_dma_start(
                    out=out_ap, out_offset=None, in_=in_tensor,
                    in_offset=bass.IndirectOffsetOnAxis(ap=idx_ap, axis=0),
                )
                if NQ > 1:
                    q = _qn[0] % NQ
                    _qn[0] += 1
                    if q:
                        inst.ins.queue = f"qPoolDynamic{q}"
                return inst

            wt_sb = cpool.tile([IN, F], dt.float32)
            nc.sync.dma_start(out=wt_sb[:], in_=wt[:])
            bb_sb = cpool.tile([128, F], dt.float32)
            nc.sync.dma_start(out=bb_sb[:], in_=bb[:])
            id128_sb = cpool.tile([128, 128], dt.float32)
            nc.sync.dma_start(out=id128_sb[:], in_=id128[:])
            id64_sb = cpool.tile([64, 64], dt.float32)
            nc.sync.dma_start(out=id64_sb[:], in_=id64[:])
            iota_sb = cpool.tile([128, TB * K], dt.float32)
            nc.sync.dma_start(out=iota_sb[:], in_=iota[:])
            cidx_sb = cpool.tile([128, GROUPS], dt.int32)
            nc.sync.dma_start(out=cidx_sb[:], in_=cidxT[:])
            ideg_sb = cpool.tile([128, GROUPS], dt.float32)
            nc.sync.dma_start(out=ideg_sb[:], in_=idegT[:])

            # ---- h0 = x @ W.T  (bias folded to the end) ----
            for g in range(GROUPS):
                xt = tpool.tile([128, IN], dt.float32, name=f"xt{g}", tag="xt")
                nc.sync.dma_start(out=xt[:], in_=xsh[g * 128:(g + 1) * 128])
                xps = qpool.tile([128, 128], dt.float32, name=f"xps{g}", tag="h0ps",
                                 space="PSUM")
                nc.tensor.transpose(out=xps[:], in_=xt[:], identity=id128_sb[:])
                xts = tpool.tile([128, 128], dt.float32, name=f"xts{g}", tag="xts")
                nc.vector.tensor_copy(out=xts[:], in_=xps[:])
                hps = qpool.tile([128, 128], dt.float32, name=f"hps{g}", tag="h0ps",
                                 space="PSUM")
                nc.tensor.matmul(out=hps[:, :F], lhsT=xts[:], rhs=wt_sb[:],
                                 start=True, stop=True)
                hsb = tpool.tile([128, F], dt.float32, name=f"hsb{g}", tag="hsb")
                nc.vector.tensor_copy(out=hsb[:], in_=hps[:, :F])
                nc.sync.dma_start(out=h_loc[0][g * 128:(g + 1) * 128], in_=hsb[:])

            def allgather(i):
                nc.gpsimd.collective_compute(
                    "AllGather",
                    mybir.AluOpType.bypass,
                    ins=[h_loc[i].ap()],
                    outs=[h_glob[i].ap()],
                    replica_groups=[list(range(NCORES))],
                )

            def hop(i, _r=0):
                """h_glob[i] -> slots[i]"""
                for k in range(nchunk):
                    if i != 0:
                        gix = ipool.tile([128, TB], dt.int32, name=f"gix{i}r{_r}_{k}", tag="gix")
                        nc.sync.dma_start(out=gix[:], in_=gidxT[k])
                    sgt = ipool.tile([128, TB], dt.float32, name=f"sgt{i}r{_r}_{k}", tag="sgt")
                    nc.sync.dma_start(out=sgt[:], in_=segT[k])
                    sel = spool.tile([128, TB * K], dt.float32, name=f"sel{i}r{_r}_{k}", tag="sel")
                    nc.vector.tensor_tensor(
                        out=sel[:].rearrange("p (t o) -> p t o", o=K),
                        in0=sgt[:].rearrange("p (t o) -> p t o", o=1).to_broadcast([128, TB, K]),
                        in1=iota_sb[:].rearrange("p (t o) -> p t o", o=K),
                        op=mybir.AluOpType.is_equal,
                    )
                    bank = ppool.tile([64, TB * K], dt.float32, name=f"bank{i}r{_r}_{k}",
                                      tag="bank", space="PSUM")
                    if i == 0:
                        # raw-x aggregation: bankX[f, slot] += x_e[f] * Sel[e, slot]
                        xgc = xgpool.tile([128, TB * IN], dt.float32,
                                         name=f"xgc{_r}_{k}", tag="xgc")
                        nc.sync.dma_start(out=xgc[:], in_=xg[k])
                        bankx = xpool.tile([128, TB * K], dt.float32,
                                           name=f"bankx{_r}_{k}", tag="bankx",
                                           space="PSUM")
                        for j in range(TB):
                            nc.tensor.matmul(
                                out=bankx[:, j * K:(j + 1) * K],
                                lhsT=xgc[:, j * IN:(j + 1) * IN],
                                rhs=sel[:, j * K:(j + 1) * K],
                                start=True, stop=True,
                            )
                        bxs = tpool.tile([128, TB * K], dt.float32,
                                         name=f"bxs{_r}_{k}", tag="bxs")
                        nc.vector.tensor_copy(out=bxs[:], in_=bankx[:])
                        # project with W once per bank: bank[o, slot] = Wt^T @ bxs
                        nc.tensor.matmul(out=bank[:], lhsT=wt_sb[:], rhs=bxs[:],
                                         start=True, stop=True)
                    else:
                        for j in range(TB):
                            msg = mpool.tile([128, F], dt.float32,
                                             name=f"msg{i}r{_r}_{k}_{j}", tag="msg")
                            _indirect(msg[:], h_glob[i][:], gix[:, j:j + 1])
                            nc.tensor.matmul(
                                out=bank[:, j * K:(j + 1) * K],
                                lhsT=msg[:],
                                rhs=sel[:, j * K:(j + 1) * K],
                                start=True, stop=True,
                            )
                    bsb = tpool.tile([64, TB * K], dt.float32, name=f"bsb{i}r{_r}_{k}", tag="bsb")
                    nc.vector.tensor_copy(out=bsb[:], in_=bank[:])
                    for q in range(TB * K // 128):
                        tps = qpool.tile([128, 64], dt.float32, name=f"tps{i}r{_r}_{k}_{q}",
                                         tag="tps", space="PSUM")
                        nc.tensor.transpose(out=tps[:], in_=bsb[:, q * 128:(q + 1) * 128],
                                            identity=id64_sb[:])
                        tsb = tpool.tile([128, 64], dt.float32, name=f"tsb{i}r{_r}_{k}_{q}", tag="tsb")
                        nc.vector.tensor_copy(out=tsb[:], in_=tps[:])
                        nc.sync.dma_start(
                            out=slots[i][k * TB * K + q * 128:
                                         k * TB * K + (q + 1) * 128],
                            in_=tsb[:])

            def compact(i, _r=0):
                """slots[i] -> h_loc[i+1] (scale by 1/deg), or out (+bias)."""
                last = i == NUM_LAYERS - 1
                hin = h_loc[i] if i == 0 else h_loc[i]
                for g in range(GROUPS):
                    crow = tpool.tile([128, F], dt.float32, name=f"crow{i}r{_r}_{g}", tag="crow")
                    _indirect(crow[:], slots[i][:], cidx_sb[:, g:g + 1])
                    own = tpool.tile([128, F], dt.float32, name=f"own{i}r{_r}_{g}", tag="own")
                    nc.sync.dma_start(out=own[:], in_=hin[g * 128:(g + 1) * 128])
                    nc.vector.tensor_tensor(out=crow[:], in0=crow[:], in1=own[:],
                                            op=mybir.AluOpType.add)
                    nc.vector.tensor_scalar(
                        out=crow[:], in0=crow[:],
                        scalar1=ideg_sb[:, g:g + 1], scalar2=None,
                        op0=mybir.AluOpType.mult,
                    )
                    if last:
                        nc.vector.tensor_tensor(out=crow[:], in0=crow[:],
                                                in1=bb_sb[:],
                                                op=mybir.AluOpType.add)
                        nc.sync.dma_start(out=out[g * 128:(g + 1) * 128], in_=crow[:])
                    else:
                        nc.sync.dma_start(out=h_loc[i + 1][g * 128:(g + 1) * 128],
                                          in_=crow[:])

            for _r in range(repeat):
                hop(0, _r)
                compact(0, _r)
                allgather(1)
                hop(1, _r)
                compact(1, _r)

    return nc


# ----------------------------------------------------------------------------
# entry point
# ----------------------------------------------------------------------------



def _make_in_maps(x, W, b, cores):
    x = np.asarray(x, np.float32)
    W = np.asarray(W, np.float32)
    b = np.asarray(b, np.float32)
    iota = np.tile(np.arange(K, dtype=np.float32), (128, TB))
    id128 = np.eye(128, dtype=np.float32)
    id64 = np.eye(64, dtype=np.float32)
    bbc = np.tile(b[None, :], (128, 1))
    in_maps = []
    for c in range(NCORES):
        xs = np.zeros((SHARD_PAD, IN), np.float32)
        xs[:SHARD] = x[c * SHARD:(c + 1) * SHARD]
        g = cores[c]["gsrc"]                       # [tpad, P] source node ids
        nchunk = cores[c]["gidxT"].shape[0]
        xg = x[g.reshape(nchunk, TB, P)]           # [nchunk, TB, P, IN]
        xg = np.ascontiguousarray(xg.transpose(0, 2, 1, 3))  # [nchunk, P, TB, IN]
        in_maps.append({
            "xg": xg,
            "xsh": xs,
            "wt": W.T.copy(),
            "bb": bbc,
            "id128": id128,
            "id64": id64,
            "iota": iota,
            "gidxT": cores[c]["gidxT"],
            "segT": cores[c]["segT"],
            "cidxT": cores[c]["cidxT"],
            "idegT": cores[c]["idegT"],
        })
    return in_maps

def kernel(x, W, b, edge_index):
    from concourse import bass_utils

    x = np.asarray(x, np.float32)
    W = np.asarray(W, np.float32)
    b = np.asarray(b, np.float32)
    edge_index = np.asarray(edge_index)

    cores, nchunk = _prepare(edge_index)
    nc = _build(nchunk)

    in_maps = _make_in_maps(x, W, b, cores)

    res = bass_utils.run_bass_kernel_spmd(nc, in_maps, core_ids=list(range(NCORES)))
    outp = np.concatenate([res.results[c]["out"][:SHARD] for c in range(NCORES)],
                          axis=0)
    return outp.astype(np.float32)


if __name__ == "__main__":
    import importlib.util
    spec = importlib.util.spec_from_file_location("refmod", "/root/problem/reference.py")
    ref = importlib.util.module_from_spec(spec)
    spec.loader.exec_module(ref)
    inputs = {k: np.asarray(v) for k, v in ref.setup_inputs().items()}
    got = kernel(**inputs)
    print("kernel output", got.shape, got.dtype)
